# revision 5
# baseline (speedup 1.0000x reference)
"""Causal self-attention (GQA, partial RoPE, qk rms-norm, logit softcap) on 8 trn2 cores.

Sharding: 8 cores = batch(2) x kv_head(4). Each core computes, for its (b, h):
  - q/k/v projections for its 4 q-heads / 1 kv-head (x @ W.T slices)
  - rms-norm, partial rope, q_gain, causal softcapped attention
  - partial output projection against Wproj columns [512h:512h+512]
Host sums the 4 partials per batch.

v3: QKV projection in fp32r (full-rate at N>=256); q/k/v, probabilities, yT,
Wproj in bf16. Softmax denominator via an all-ones [128,128] stationary matmul
(broadcast denominator straight into PSUM) with reciprocal_approx_fast.
rms sqrt is batched per chunk so the ACT table doesn't thrash between
sqrt and tanh/exp. Phase-1 stages and deferred output-projection units are
interleaved into the attention pair loop as PE filler; evacuations trail
their producers by one slot to avoid head-of-line waits on in-order queues.
"""
import math
import numpy as np
from contextlib import ExitStack

import ml_dtypes

import concourse.bass as bass
import concourse.tile as tile
from concourse import bacc, mybir
from concourse.bass_utils import run_bass_kernel_spmd
from concourse.alu_op_type import AluOpType

F32 = mybir.dt.float32
F32R = mybir.dt.float32r
BF16 = mybir.dt.bfloat16

B = 2
S = 2048
D = 2048
H = 16
HKV = 4
HD = 128
G = 4  # q heads per kv head (= heads per core)
ROPE = 32
HALF = ROPE // 2  # 16
ROPE_BASE = 10000.0
CAP = 30.0
EPS = float(np.finfo(np.float32).eps)
NST = S // 128  # 16 s-tiles
NCH = S // 512  # 4 sq chunks
NDT = D // 128  # 16 d k-tiles
FQKV = G * HD + 2 * HD  # 768

_CACHE = {}


def _build():
    nc = bacc.Bacc("TRN2", target_bir_lowering=False, debug=False)

    xT = nc.dram_tensor("xT", [D, S], F32R, kind="ExternalInput").ap()
    wqkv = nc.dram_tensor("wqkv", [D, FQKV], F32R, kind="ExternalInput").ap()
    wpT = nc.dram_tensor("wpT", [G * HD, D], BF16, kind="ExternalInput").ap()
    gains = nc.dram_tensor("gains", [128, G], F32, kind="ExternalInput").ap()
    cos4 = nc.dram_tensor("cos4", [S, G * HALF], F32, kind="ExternalInput").ap()
    sin4 = nc.dram_tensor("sin4", [S, G * HALF], F32, kind="ExternalInput").ap()
    out = nc.dram_tensor("out", [S, D], F32, kind="ExternalOutput").ap()

    xT_r = xT.rearrange("(dt p) s -> p dt s", p=128)       # [128, 16, 2048]
    wqkv_r = wqkv.rearrange("(dt p) f -> p dt f", p=128)   # [128, 16, 768]
    wpT_r = wpT.rearrange("(g p) j -> p g j", p=128)       # [128, 4, 2048]
    cos_r = cos4.rearrange("(t p) f -> p t f", p=128)      # [128, 16, 64]
    sin_r = sin4.rearrange("(t p) f -> p t f", p=128)
    out_r = out.rearrange("(t p) j -> t p j", p=128)       # [16, 128, 2048]

    with tile.TileContext(nc) as tc:
        with ExitStack() as ctx:
            persist = ctx.enter_context(tc.tile_pool(name="persist", bufs=1))

            # ---------------- pools ----------------
            xc_pool = ctx.enter_context(tc.tile_pool(name="xc", bufs=3))
            qr_pool = ctx.enter_context(tc.tile_pool(name="qr", bufs=6))
            kr_pool = ctx.enter_context(tc.tile_pool(name="kr", bufs=6))
            p1s = ctx.enter_context(tc.tile_pool(name="p1s", bufs=2))
            tb_pool = ctx.enter_context(tc.tile_pool(name="tb", bufs=2))
            rc_pool = ctx.enter_context(tc.tile_pool(name="rc", bufs=2))
            osb_pool = ctx.enter_context(tc.tile_pool(name="osb", bufs=3))
            psq_pool = ctx.enter_context(tc.tile_pool(name="psq", bufs=1, space="PSUM"))
            pskv_pool = ctx.enter_context(tc.tile_pool(name="pskv", bufs=1, space="PSUM"))
            ptr_pool = ctx.enter_context(tc.tile_pool(name="ptr", bufs=1, space="PSUM"))
            pss_pool = ctx.enter_context(tc.tile_pool(name="pss", bufs=1, space="PSUM"))
            psy_pool = ctx.enter_context(tc.tile_pool(name="psy", bufs=1, space="PSUM"))
            psd_pool = ctx.enter_context(tc.tile_pool(name="psd", bufs=1, space="PSUM"))
            pso_pool = ctx.enter_context(tc.tile_pool(name="pso", bufs=1, space="PSUM"))

            # ---- DMA priority order: x tile 0, then wqkv (split across the
            # two hw queues), then rope tables, wpT last. ----
            xc_tiles = {}

            def prefetch_xc(st):
                if st < NST and st not in xc_tiles:
                    t = xc_pool.tile([128, NDT, 128], F32R, tag="xc")
                    nc.sync.dma_start(out=t, in_=xT_r[:, :, st * 128:(st + 1) * 128])
                    xc_tiles[st] = t

            wqkv_sb = persist.tile([128, NDT, FQKV], F32R)
            wpT_sb = persist.tile([128, G, D], BF16)
            gains_sb = persist.tile([128, G], F32)
            cos_all = persist.tile([128, NST, G * HALF], F32)
            sin_all = persist.tile([128, NST, G * HALF], F32)

            prefetch_xc(0)
            for dt in range(NDT):
                eng = nc.sync if dt % 2 == 0 else nc.scalar
                eng.dma_start(out=wqkv_sb[:, dt, :], in_=wqkv_r[:, dt, :])
            prefetch_xc(1)
            nc.scalar.dma_start(out=gains_sb, in_=gains)
            nc.scalar.dma_start(out=cos_all, in_=cos_r)
            nc.scalar.dma_start(out=sin_all, in_=sin_r)
            nc.scalar.dma_start(out=wpT_sb, in_=wpT_r)

            # ---- constants / persistent tensors ----
            ident_f = persist.tile([128, 128], F32)
            nc.gpsimd.memset(ident_f, 0.0)
            nc.gpsimd.affine_select(
                out=ident_f, in_=ident_f, compare_op=AluOpType.not_equal,
                fill=1.0, base=0, pattern=[[-1, 128]], channel_multiplier=1,
            )
            ident = persist.tile([128, 128], BF16)
            nc.vector.tensor_copy(ident, ident_f)

            ones_sq = persist.tile([128, 128], BF16)
            nc.vector.memset(ones_sq, 1.0)

            # diagonal-block 0/1 masks (r = kb - 4c in 0..3): valid iff sq >= r*128 + sk
            masks = persist.tile([128, 4, 512], BF16)
            mask_f = persist.tile([128, 512], F32)
            for r in range(4):
                nc.vector.memset(mask_f, 1.0)
                nc.gpsimd.affine_select(
                    out=mask_f, in_=mask_f, compare_op=AluOpType.is_ge,
                    fill=0.0, base=-128 * r, pattern=[[1, 512]], channel_multiplier=-1,
                )
                nc.vector.tensor_copy(masks[:, r, :], mask_f)

            eps_t = persist.tile([128, 1], F32)
            nc.vector.memset(eps_t, EPS)

            qT_all = persist.tile([128, G, S], BF16)   # [f, g, s]
            kT_all = persist.tile([128, S], BF16)      # [f, s]
            v_all = persist.tile([128, NST, HD], BF16)  # [sk within tile, st, f]
            yT_all = persist.tile([128, G, S], BF16)   # [f, g, s]
            ms_all = persist.tile([128, NST, 5], F32)  # rms sum-sq per st (4 q heads + k)
            rstd_all = persist.tile([128, NST, 5], F32)
            gsc_all = persist.tile([128, NST, G], F32)
            qsc = persist.tile([128, 1], F32)          # scratch scalar for ttr

            # probability double-buffer; trimmed head regions pre-zeroed once
            # (later uses of a block slot always write a superset range).
            p_bufs = [persist.tile([128, NST, 512], BF16, name=f"pbuf{i}")
                      for i in range(2)]
            for pb in p_bufs:
                for kb in range(NST):
                    r = kb % 4
                    if r:
                        nc.gpsimd.memset(pb[:, kb, 0:128 * r], 0.0)

            # ---- phase 1 stages (emitted as PE filler inside attention) ----
            p1_state = {}

            def p1_mm(st):
                """QKV projection matmuls for one s-tile."""
                xc = xc_tiles.pop(st)
                psq = psq_pool.tile([128, G * HD], F32, tag="psq")
                for dt in range(NDT):
                    nc.tensor.matmul(psq, xc[:, dt, :], wqkv_sb[:, dt, 0:G * HD],
                                     start=(dt == 0), stop=(dt == NDT - 1))
                pskv = pskv_pool.tile([128, 2 * HD], F32, tag="pskv")
                for dt in range(NDT):
                    nc.tensor.matmul(pskv, xc[:, dt, :], wqkv_sb[:, dt, G * HD:FQKV],
                                     start=(dt == 0), stop=(dt == NDT - 1))
                p1_state[st] = (psq, pskv)

            def p1_rope(st):
                """rms sum-squares (fused mul+reduce) + unscaled rope; v evac."""
                psq, pskv = p1_state.pop(st)
                sq2 = p1s.tile([128, G * HD], F32, tag="sq2")
                sk2 = p1s.tile([128, HD], F32, tag="sk2")
                psq_v = psq.rearrange("p (g d) -> p g d", g=G)
                nc.scalar.activation(sq2, psq, mybir.ActivationFunctionType.Square)
                nc.scalar.activation(sk2, pskv[:, 0:HD],
                                     mybir.ActivationFunctionType.Square)
                nc.vector.reduce_sum(ms_all[:, st, 0:4],
                                     sq2.rearrange("p (g d) -> p g d", g=G),
                                     axis=mybir.AxisListType.X)
                nc.vector.reduce_sum(ms_all[:, st, 4:5], sk2,
                                     axis=mybir.AxisListType.X)

                cos_t = cos_all[:, st, :].rearrange("p (g d) -> p g d", g=G)
                sin_t = sin_all[:, st, :].rearrange("p (g d) -> p g d", g=G)

                q_rot = qr_pool.tile([128, G, HD], BF16, tag="q_rot")
                tmp = p1s.tile([128, G, HALF], F32, tag="tmp")
                qa = p1s.tile([128, G, HALF], F32, tag="qa")
                nc.vector.tensor_mul(qa, psq_v[:, :, 0:HALF], cos_t)
                nc.vector.tensor_mul(tmp, psq_v[:, :, HALF:ROPE], sin_t)
                nc.vector.tensor_add(q_rot[:, :, 0:HALF], qa, tmp)
                nc.vector.tensor_mul(qa, psq_v[:, :, HALF:ROPE], cos_t)
                nc.vector.tensor_mul(tmp, psq_v[:, :, 0:HALF], sin_t)
                nc.vector.tensor_sub(q_rot[:, :, HALF:ROPE], qa, tmp)
                nc.vector.tensor_copy(q_rot[:, :, ROPE:HD], psq_v[:, :, ROPE:HD])

                k_rot = kr_pool.tile([128, HD], BF16, tag="k_rot")
                ktmp = p1s.tile([128, HALF], F32, tag="ktmp")
                ka = p1s.tile([128, HALF], F32, tag="ka")
                kc = cos_all[:, st, 0:HALF]
                ks = sin_all[:, st, 0:HALF]
                nc.vector.tensor_mul(ka, pskv[:, 0:HALF], kc)
                nc.vector.tensor_mul(ktmp, pskv[:, HALF:ROPE], ks)
                nc.vector.tensor_add(k_rot[:, 0:HALF], ka, ktmp)
                nc.vector.tensor_mul(ka, pskv[:, HALF:ROPE], kc)
                nc.vector.tensor_mul(ktmp, pskv[:, 0:HALF], ks)
                nc.vector.tensor_sub(k_rot[:, HALF:ROPE], ka, ktmp)
                nc.vector.tensor_copy(k_rot[:, ROPE:HD], pskv[:, ROPE:HD])
                nc.vector.tensor_copy(v_all[:, st, :], pskv[:, HD:2 * HD])
                p1_state[st] = (q_rot, k_rot)

            def p1_finish(st_lo, n_st):
                """Batched rstd (one ACT sqrt -> table loaded once per chunk),
                scales on gpsimd, transposes + evacuations."""
                nc.scalar.activation(rstd_all[:, st_lo:st_lo + n_st, :],
                                     ms_all[:, st_lo:st_lo + n_st, :],
                                     mybir.ActivationFunctionType.Sqrt,
                                     scale=1.0 / HD, bias=eps_t)
                nc.vector.reciprocal(rstd_all[:, st_lo:st_lo + n_st, :],
                                     rstd_all[:, st_lo:st_lo + n_st, :])
                for st in range(st_lo, st_lo + n_st):
                    nc.vector.tensor_mul(gsc_all[:, st, :], rstd_all[:, st, 0:4],
                                         gains_sb)
                for st in range(st_lo, st_lo + n_st):
                    q_rot, k_rot = p1_state.pop(st)
                    for h in range(G):
                        nc.gpsimd.tensor_scalar_mul(q_rot[:, h, :], q_rot[:, h, :],
                                                    gsc_all[:, st, h:h + 1])
                    nc.gpsimd.tensor_scalar_mul(k_rot, k_rot, rstd_all[:, st, 4:5])
                    ptr = ptr_pool.tile([128, 5, 128], BF16, tag="ptr")
                    for h in range(G):
                        nc.tensor.transpose(ptr[:, h, :], q_rot[:, h, :], ident)
                    nc.tensor.transpose(ptr[:, 4, :], k_rot, ident)
                    nc.vector.tensor_copy(qT_all[:, :, st * 128:(st + 1) * 128],
                                          ptr[:, 0:4, :])
                    nc.vector.tensor_copy(kT_all[:, st * 128:(st + 1) * 128],
                                          ptr[:, 4, :])

            # filler scheduling state
            stage_q = []        # pending phase-1 stage closures
            proj_q = []         # pending (st, jc) output-projection units
            evac_q = []         # deferred proj evacuations (ps_o, st, jc, eng)
            filler_ctr = [0]

            def proj_mm(st, jc):
                ps_o = pso_pool.tile([128, 512], F32, tag="pso")
                for g4 in range(G):
                    nc.tensor.matmul(
                        ps_o,
                        yT_all[:, g4, st * 128:(st + 1) * 128],
                        wpT_sb[:, g4, jc * 512:(jc + 1) * 512],
                        start=(g4 == 0), stop=(g4 == G - 1),
                    )
                eng = "v" if filler_ctr[0] % 2 == 0 else "s"
                filler_ctr[0] += 1
                evac_q.append((ps_o, st, jc, eng))

            def proj_evac():
                ps_o, st, jc, eng = evac_q.pop(0)
                o_sb = osb_pool.tile([128, 512], F32, tag="osb")
                if eng == "v":
                    nc.vector.tensor_copy(o_sb, ps_o)
                else:
                    nc.scalar.copy(o_sb, ps_o)
                nc.sync.dma_start(out=out_r[st][:, jc * 512:(jc + 1) * 512], in_=o_sb)

            def emit_filler():
                # deferred evac first (deps long resolved), then one PE unit
                if evac_q:
                    proj_evac()
                if stage_q:
                    stage_q.pop(0)()
                elif proj_q:
                    st, jc = proj_q.pop(0)
                    proj_mm(st, jc)

            # ---------------- prologue: phase 1 for st 0..3 ----------------
            for st in range(4):
                p1_mm(st)
                p1_rope(st)
                prefetch_xc(st + 2)
            p1_finish(0, 4)

            # one-time zero of the score PSUM slot so full-range tanh reads
            # finite values even where the trimmed matmuls never write
            pss0 = pss_pool.tile([128, 2, 512], F32, tag="pss")
            nc.vector.memset(pss0, 0.0)

            # ---------------- main loop: attention + interleaved work ------
            for c in range(NCH):
                nkv = 4 * (c + 1)
                for g in range(G):
                    st_next = 4 * (c + 1) + g
                    if st_next < NST:
                        stage_q.append(lambda st=st_next: p1_mm(st))
                        stage_q.append(lambda st=st_next: p1_rope(st))
                    prefetch_xc(st_next + 1)

                    qT_c = qT_all[:, g, c * 512:(c + 1) * 512]
                    p_buf = p_bufs[(4 * c + g) % 2]
                    ps_y = psy_pool.tile([128, 512], F32, tag="psy")
                    ps_d = psd_pool.tile([128, 512], F32, tag="psd")

                    pairs = list(range(0, nkv, 2))
                    prev = None  # previous pair's (kb0, off_pair)
                    for kb0 in pairs:
                        offs = [max(0, 128 * (kb0 + i) - 512 * c) for i in (0, 1)]
                        off_p = offs[0]
                        ps_s = pss_pool.tile([128, 2, 512], F32, tag="pss")
                        for i in (0, 1):
                            kb = kb0 + i
                            nc.tensor.matmul(
                                ps_s[:, i, offs[i]:512],
                                kT_all[:, kb * 128:(kb + 1) * 128],
                                qT_c[:, offs[i]:512],
                                start=True, stop=True,
                            )
                        # V/d of the previous pair (deps resolved -> no stall)
                        if prev is not None:
                            for kb in (prev, prev + 1):
                                nc.tensor.matmul(ps_y, v_all[:, kb, :], p_buf[:, kb, :],
                                                 start=(kb == 0), stop=False)
                                nc.tensor.matmul(ps_d, ones_sq, p_buf[:, kb, :],
                                                 start=(kb == 0), stop=False)
                        emit_filler()
                        if kb0 == 0:
                            emit_filler()

                        # full-range tanh (stale PSUM is finite), exp from the
                        # pair's min offset; masks zero everything invalid
                        t_b = tb_pool.tile([128, 2, 512], F32, tag="tb")
                        nc.scalar.activation(t_b, ps_s,
                                             mybir.ActivationFunctionType.Tanh,
                                             scale=1.0 / CAP)
                        nc.scalar.activation(p_buf[:, kb0:kb0 + 2, off_p:512],
                                             t_b[:, :, off_p:512],
                                             mybir.ActivationFunctionType.Exp,
                                             scale=CAP)
                        for i in (0, 1):
                            kb = kb0 + i
                            r = kb - 4 * c
                            if r >= 0:
                                nc.gpsimd.tensor_mul(p_buf[:, kb, off_p:512],
                                                     p_buf[:, kb, off_p:512],
                                                     masks[:, r, off_p:512])
                        prev = kb0

                    # tail: V/d of the last pair, then normalize
                    for kb in (prev, prev + 1):
                        nc.tensor.matmul(ps_y, v_all[:, kb, :], p_buf[:, kb, :],
                                         start=(kb == 0), stop=(kb == nkv - 1))
                        nc.tensor.matmul(ps_d, ones_sq, p_buf[:, kb, :],
                                         start=(kb == 0), stop=(kb == nkv - 1))
                    recip = rc_pool.tile([128, 512], F32, tag="rc")
                    nc.vector.reciprocal_approx_fast(out=recip, in_=ps_d)
                    nc.vector.tensor_mul(yT_all[:, g, c * 512:(c + 1) * 512],
                                         ps_y, recip)

                # finish phase-1 for the next chunk's s-tiles (batched sqrt,
                # scales, transposes) before its attention needs qT/kT
                while stage_q:
                    stage_q.pop(0)()
                if c + 1 < NCH:
                    p1_finish(4 * (c + 1), 4)

                # queue output projection for this chunk (runs as filler later)
                for st in range(4 * c, 4 * c + 4):
                    for jc in range(4):
                        proj_q.append((st, jc))

            # drain remaining projection units
            while proj_q or evac_q:
                emit_filler()

    nc.compile()
    return nc


def _host_prep(x, Wq, Wk, Wv, Wproj, q_gain):
    inv_freq = 1.0 / (ROPE_BASE ** (np.arange(0, ROPE, 2, dtype=np.float32) / ROPE))
    t = np.arange(S, dtype=np.float32)
    freqs = np.outer(t, inv_freq).astype(np.float32)  # [S, 16]
    cos = np.cos(freqs).astype(np.float32)
    sin = np.sin(freqs).astype(np.float32)
    cos4 = np.ascontiguousarray(np.tile(cos[:, None, :], (1, G, 1)).reshape(S, G * HALF))
    sin4 = np.ascontiguousarray(np.tile(sin[:, None, :], (1, G, 1)).reshape(S, G * HALF))

    xT = [np.ascontiguousarray(x[b].T) for b in range(B)]

    in_maps = []
    for core in range(8):
        b, h = core // HKV, core % HKV
        wqkv = np.ascontiguousarray(
            np.concatenate(
                [Wq[512 * h:512 * h + 512].T,
                 Wk[128 * h:128 * h + 128].T,
                 Wv[128 * h:128 * h + 128].T], axis=1
            )
        )
        wpT = np.ascontiguousarray(Wproj[:, 512 * h:512 * h + 512].T)
        gains = np.ascontiguousarray(
            np.broadcast_to((q_gain[G * h:G * h + G] / math.sqrt(HD)).astype(np.float32)[None, :],
                            (128, G))
        )
        in_maps.append({
            "xT": xT[b],
            "wqkv": wqkv.astype(np.float32),
            "wpT": wpT.astype(ml_dtypes.bfloat16),
            "gains": gains,
            "cos4": cos4,
            "sin4": sin4,
        })
    return in_maps


def kernel(x, Wq, Wk, Wv, Wproj, q_gain, _trace=False):
    x = np.asarray(x, dtype=np.float32)
    Wq = np.asarray(Wq, dtype=np.float32)
    Wk = np.asarray(Wk, dtype=np.float32)
    Wv = np.asarray(Wv, dtype=np.float32)
    Wproj = np.asarray(Wproj, dtype=np.float32)
    q_gain = np.asarray(q_gain, dtype=np.float32)

    if "nc" not in _CACHE:
        _CACHE["nc"] = _build()
    nc = _CACHE["nc"]

    in_maps = _host_prep(x, Wq, Wk, Wv, Wproj, q_gain)
    res = run_bass_kernel_spmd(nc, in_maps, core_ids=list(range(8)), trace=_trace)

    out = np.empty((B, S, D), dtype=np.float32)
    for b in range(B):
        acc = np.zeros((S, D), dtype=np.float64)
        for h in range(HKV):
            acc += res.results[b * HKV + h]["out"]
        out[b] = acc.astype(np.float32)
    if _trace:
        return out, res
    return out


# revision 6
# speedup vs baseline: 1.4239x; 1.4239x over previous
"""Causal self-attention (GQA, partial RoPE, qk rms-norm, logit softcap) on 8 trn2 cores.

Sharding: 8 cores = batch(2) x kv_head(4). Each core computes, for its (b, h):
  - q/k/v projections for its 4 q-heads / 1 kv-head (x @ W.T slices)
  - rms-norm, partial rope, q_gain, causal softcapped attention
  - partial output projection against Wproj columns [512h:512h+512]
Host sums the 4 partials per batch.

v3: QKV projection in fp32r (full-rate at N>=256); q/k/v, probabilities, yT,
Wproj in bf16. Softmax denominator via an all-ones [128,128] stationary matmul
(broadcast denominator straight into PSUM) with reciprocal_approx_fast.
rms sqrt is batched per chunk so the ACT table doesn't thrash between
sqrt and tanh/exp. Phase-1 stages and deferred output-projection units are
interleaved into the attention pair loop as PE filler; evacuations trail
their producers by one slot to avoid head-of-line waits on in-order queues.
"""
import math
import numpy as np
from contextlib import ExitStack

import ml_dtypes

import concourse.bass as bass
import concourse.tile as tile
from concourse import bacc, mybir
from concourse.bass_utils import run_bass_kernel_spmd
from concourse.alu_op_type import AluOpType

F32 = mybir.dt.float32
F32R = mybir.dt.float32r
BF16 = mybir.dt.bfloat16

B = 2
S = 2048
D = 2048
H = 16
HKV = 4
HD = 128
G = 4  # q heads per kv head (= heads per core)
ROPE = 32
HALF = ROPE // 2  # 16
ROPE_BASE = 10000.0
CAP = 30.0
EPS = float(np.finfo(np.float32).eps)
NST = S // 128  # 16 s-tiles
NCH = S // 512  # 4 sq chunks
NDT = D // 128  # 16 d k-tiles
FQKV = G * HD + 2 * HD  # 768

_CACHE = {}


def _build():
    nc = bacc.Bacc("TRN2", target_bir_lowering=False, debug=False)

    xT = nc.dram_tensor("xT", [D, S], F32R, kind="ExternalInput").ap()
    wqkv = nc.dram_tensor("wqkv", [D, FQKV], F32R, kind="ExternalInput").ap()
    wpT = nc.dram_tensor("wpT", [G * HD, D], BF16, kind="ExternalInput").ap()
    gains = nc.dram_tensor("gains", [128, G], F32, kind="ExternalInput").ap()
    cos4 = nc.dram_tensor("cos4", [S, G * HALF], F32, kind="ExternalInput").ap()
    sin4 = nc.dram_tensor("sin4", [S, G * HALF], F32, kind="ExternalInput").ap()
    out = nc.dram_tensor("out", [S, D], F32, kind="ExternalOutput").ap()

    xT_r = xT.rearrange("(dt p) s -> p dt s", p=128)       # [128, 16, 2048]
    wqkv_r = wqkv.rearrange("(dt p) f -> p dt f", p=128)   # [128, 16, 768]
    wpT_r = wpT.rearrange("(g p) j -> p g j", p=128)       # [128, 4, 2048]
    cos_r = cos4.rearrange("(t p) f -> p t f", p=128)      # [128, 16, 64]
    sin_r = sin4.rearrange("(t p) f -> p t f", p=128)
    out_r = out.rearrange("(t p) j -> t p j", p=128)       # [16, 128, 2048]

    with tile.TileContext(nc) as tc:
        with ExitStack() as ctx:
            persist = ctx.enter_context(tc.tile_pool(name="persist", bufs=1))

            # ---------------- pools ----------------
            xc_pool = ctx.enter_context(tc.tile_pool(name="xc", bufs=3))
            qr_pool = ctx.enter_context(tc.tile_pool(name="qr", bufs=6))
            kr_pool = ctx.enter_context(tc.tile_pool(name="kr", bufs=6))
            p1s = ctx.enter_context(tc.tile_pool(name="p1s", bufs=2))
            tb_pool = ctx.enter_context(tc.tile_pool(name="tb", bufs=2))
            rc_pool = ctx.enter_context(tc.tile_pool(name="rc", bufs=2))
            osb_pool = ctx.enter_context(tc.tile_pool(name="osb", bufs=3))
            psq_pool = ctx.enter_context(tc.tile_pool(name="psq", bufs=1, space="PSUM"))
            pskv_pool = ctx.enter_context(tc.tile_pool(name="pskv", bufs=1, space="PSUM"))
            ptr_pool = ctx.enter_context(tc.tile_pool(name="ptr", bufs=1, space="PSUM"))
            pss_pool = ctx.enter_context(tc.tile_pool(name="pss", bufs=1, space="PSUM"))
            psy_pool = ctx.enter_context(tc.tile_pool(name="psy", bufs=1, space="PSUM"))
            psd_pool = ctx.enter_context(tc.tile_pool(name="psd", bufs=1, space="PSUM"))
            pso_pool = ctx.enter_context(tc.tile_pool(name="pso", bufs=1, space="PSUM"))

            # ---- DMA priority order: x tile 0, then wqkv (split across the
            # two hw queues), then rope tables, wpT last. ----
            xc_tiles = {}

            def prefetch_xc(st):
                if st < NST and st not in xc_tiles:
                    t = xc_pool.tile([128, NDT, 128], F32R, tag="xc")
                    nc.sync.dma_start(out=t, in_=xT_r[:, :, st * 128:(st + 1) * 128])
                    xc_tiles[st] = t

            wqkv_sb = persist.tile([128, NDT, FQKV], F32R)
            wpT_sb = persist.tile([128, G, D], BF16)
            gains_sb = persist.tile([128, G], F32)
            cos_all = persist.tile([128, NST, G * HALF], F32)
            sin_all = persist.tile([128, NST, G * HALF], F32)

            prefetch_xc(0)
            for dt in range(NDT):
                eng = nc.sync if dt % 2 == 0 else nc.scalar
                eng.dma_start(out=wqkv_sb[:, dt, :], in_=wqkv_r[:, dt, :])
            prefetch_xc(1)
            nc.scalar.dma_start(out=gains_sb, in_=gains)
            nc.scalar.dma_start(out=cos_all, in_=cos_r)
            nc.scalar.dma_start(out=sin_all, in_=sin_r)
            nc.scalar.dma_start(out=wpT_sb, in_=wpT_r)

            # ---- constants / persistent tensors ----
            ident_f = persist.tile([128, 128], F32)
            nc.gpsimd.memset(ident_f, 0.0)
            nc.gpsimd.affine_select(
                out=ident_f, in_=ident_f, compare_op=AluOpType.not_equal,
                fill=1.0, base=0, pattern=[[-1, 128]], channel_multiplier=1,
            )
            ident = persist.tile([128, 128], BF16)
            nc.vector.tensor_copy(ident, ident_f)

            ones_sq = persist.tile([128, 128], BF16)
            nc.vector.memset(ones_sq, 1.0)

            # diagonal-block 0/1 masks (r = kb - 4c in 0..3): valid iff sq >= r*128 + sk
            masks = persist.tile([128, 4, 512], BF16)
            mask_f = persist.tile([128, 512], F32)
            for r in range(4):
                nc.vector.memset(mask_f, 1.0)
                nc.gpsimd.affine_select(
                    out=mask_f, in_=mask_f, compare_op=AluOpType.is_ge,
                    fill=0.0, base=-128 * r, pattern=[[1, 512]], channel_multiplier=-1,
                )
                nc.vector.tensor_copy(masks[:, r, :], mask_f)

            eps_t = persist.tile([128, 1], F32)
            nc.vector.memset(eps_t, EPS)

            qT_all = persist.tile([128, G, S], BF16)   # [f, g, s]
            kT_all = persist.tile([128, S], BF16)      # [f, s]
            v_all = persist.tile([128, NST, HD], BF16)  # [sk within tile, st, f]
            yT_all = persist.tile([128, G, S], BF16)   # [f, g, s]
            ms_all = persist.tile([128, NST, 5], F32)  # rms sum-sq per st (4 q heads + k)
            rstd_all = persist.tile([128, NST, 5], F32)
            gsc_all = persist.tile([128, NST, G], F32)
            qsc = persist.tile([128, 1], F32)          # scratch scalar for ttr

            # probability double-buffer; trimmed head regions pre-zeroed once
            # (later uses of a block slot always write a superset range).
            p_bufs = [persist.tile([128, NST, 512], BF16, name=f"pbuf{i}")
                      for i in range(2)]
            for pb in p_bufs:
                for kb in range(NST):
                    r = kb % 4
                    if r:
                        nc.gpsimd.memset(pb[:, kb, 0:128 * r], 0.0)

            # ---- phase 1 stages (emitted as PE filler inside attention) ----
            p1_state = {}

            def p1_mm(st):
                """QKV projection matmuls for one s-tile."""
                xc = xc_tiles.pop(st)
                psq = psq_pool.tile([128, G * HD], F32, tag="psq")
                for dt in range(NDT):
                    nc.tensor.matmul(psq, xc[:, dt, :], wqkv_sb[:, dt, 0:G * HD],
                                     start=(dt == 0), stop=(dt == NDT - 1))
                pskv = pskv_pool.tile([128, 2 * HD], F32, tag="pskv")
                for dt in range(NDT):
                    nc.tensor.matmul(pskv, xc[:, dt, :], wqkv_sb[:, dt, G * HD:FQKV],
                                     start=(dt == 0), stop=(dt == NDT - 1))
                p1_state[st] = (psq, pskv)

            def p1_rope(st):
                """rms sum-squares (fused mul+reduce) + unscaled rope; v evac."""
                psq, pskv = p1_state.pop(st)
                sq2 = p1s.tile([128, G * HD], F32, tag="sq2")
                sk2 = p1s.tile([128, HD], F32, tag="sk2")
                psq_v = psq.rearrange("p (g d) -> p g d", g=G)
                nc.scalar.activation(sq2, psq, mybir.ActivationFunctionType.Square)
                nc.scalar.activation(sk2, pskv[:, 0:HD],
                                     mybir.ActivationFunctionType.Square)
                nc.vector.reduce_sum(ms_all[:, st, 0:4],
                                     sq2.rearrange("p (g d) -> p g d", g=G),
                                     axis=mybir.AxisListType.X)
                nc.vector.reduce_sum(ms_all[:, st, 4:5], sk2,
                                     axis=mybir.AxisListType.X)

                cos_t = cos_all[:, st, :].rearrange("p (g d) -> p g d", g=G)
                sin_t = sin_all[:, st, :].rearrange("p (g d) -> p g d", g=G)

                q_rot = qr_pool.tile([128, G, HD], BF16, tag="q_rot")
                tmp = p1s.tile([128, G, HALF], F32, tag="tmp")
                qa = p1s.tile([128, G, HALF], F32, tag="qa")
                nc.vector.tensor_mul(qa, psq_v[:, :, 0:HALF], cos_t)
                nc.vector.tensor_mul(tmp, psq_v[:, :, HALF:ROPE], sin_t)
                nc.vector.tensor_add(q_rot[:, :, 0:HALF], qa, tmp)
                nc.vector.tensor_mul(qa, psq_v[:, :, HALF:ROPE], cos_t)
                nc.vector.tensor_mul(tmp, psq_v[:, :, 0:HALF], sin_t)
                nc.vector.tensor_sub(q_rot[:, :, HALF:ROPE], qa, tmp)
                nc.vector.tensor_copy(q_rot[:, :, ROPE:HD], psq_v[:, :, ROPE:HD])

                k_rot = kr_pool.tile([128, HD], BF16, tag="k_rot")
                ktmp = p1s.tile([128, HALF], F32, tag="ktmp")
                ka = p1s.tile([128, HALF], F32, tag="ka")
                kc = cos_all[:, st, 0:HALF]
                ks = sin_all[:, st, 0:HALF]
                nc.vector.tensor_mul(ka, pskv[:, 0:HALF], kc)
                nc.vector.tensor_mul(ktmp, pskv[:, HALF:ROPE], ks)
                nc.vector.tensor_add(k_rot[:, 0:HALF], ka, ktmp)
                nc.vector.tensor_mul(ka, pskv[:, HALF:ROPE], kc)
                nc.vector.tensor_mul(ktmp, pskv[:, 0:HALF], ks)
                nc.vector.tensor_sub(k_rot[:, HALF:ROPE], ka, ktmp)
                nc.vector.tensor_copy(k_rot[:, ROPE:HD], pskv[:, ROPE:HD])
                nc.vector.tensor_copy(v_all[:, st, :], pskv[:, HD:2 * HD])
                p1_state[st] = (q_rot, k_rot)

            def p1_rstd(st_lo, n_st):
                """Batched rstd: one ACT sqrt per chunk so the activation
                table doesn't thrash between sqrt and tanh/exp."""
                nc.scalar.activation(rstd_all[:, st_lo:st_lo + n_st, :],
                                     ms_all[:, st_lo:st_lo + n_st, :],
                                     mybir.ActivationFunctionType.Sqrt,
                                     scale=1.0 / HD, bias=eps_t)
                nc.vector.reciprocal(rstd_all[:, st_lo:st_lo + n_st, :],
                                     rstd_all[:, st_lo:st_lo + n_st, :])
                for st in range(st_lo, st_lo + n_st):
                    nc.vector.tensor_mul(gsc_all[:, st, :], rstd_all[:, st, 0:4],
                                         gains_sb)

            def p1_tr(st):
                """scale q/k by rstd, transpose into [f, s], evacuate."""
                q_rot, k_rot = p1_state.pop(st)
                for h in range(G):
                    nc.vector.tensor_scalar_mul(q_rot[:, h, :], q_rot[:, h, :],
                                                gsc_all[:, st, h:h + 1])
                nc.vector.tensor_scalar_mul(k_rot, k_rot, rstd_all[:, st, 4:5])
                ptr = ptr_pool.tile([128, 5, 128], BF16, tag="ptr")
                for h in range(G):
                    nc.tensor.transpose(ptr[:, h, :], q_rot[:, h, :], ident)
                nc.tensor.transpose(ptr[:, 4, :], k_rot, ident)
                nc.vector.tensor_copy(qT_all[:, :, st * 128:(st + 1) * 128],
                                      ptr[:, 0:4, :])
                nc.vector.tensor_copy(kT_all[:, st * 128:(st + 1) * 128],
                                      ptr[:, 4, :])

            def p1_finish(st_lo, n_st):
                p1_rstd(st_lo, n_st)
                for st in range(st_lo, st_lo + n_st):
                    p1_tr(st)

            # filler scheduling state
            stage_q = []        # pending phase-1 stage closures
            proj_q = []         # pending (st, jc) output-projection units
            evac_q = []         # deferred proj evacuations (ps_o, st, jc, eng)
            filler_ctr = [0]

            def proj_mm(st, jc):
                ps_o = pso_pool.tile([128, 512], F32, tag="pso")
                for g4 in range(G):
                    nc.tensor.matmul(
                        ps_o,
                        yT_all[:, g4, st * 128:(st + 1) * 128],
                        wpT_sb[:, g4, jc * 512:(jc + 1) * 512],
                        start=(g4 == 0), stop=(g4 == G - 1),
                    )
                eng = "v" if filler_ctr[0] % 2 == 0 else "s"
                filler_ctr[0] += 1
                evac_q.append((ps_o, st, jc, eng))

            def proj_evac():
                ps_o, st, jc, eng = evac_q.pop(0)
                o_sb = osb_pool.tile([128, 512], F32, tag="osb")
                if eng == "v":
                    nc.vector.tensor_copy(o_sb, ps_o)
                else:
                    nc.scalar.copy(o_sb, ps_o)
                nc.sync.dma_start(out=out_r[st][:, jc * 512:(jc + 1) * 512], in_=o_sb)

            def emit_filler():
                # deferred evac first (deps long resolved), then one PE unit
                if evac_q:
                    proj_evac()
                if stage_q:
                    stage_q.pop(0)()
                elif proj_q:
                    st, jc = proj_q.pop(0)
                    proj_mm(st, jc)

            # ---------------- prologue: phase 1 for st 0..4 ----------------
            for st in range(5):
                p1_mm(st)
                p1_rope(st)
                prefetch_xc(st + 2)
            p1_finish(0, 4)

            # one-time zero of the score PSUM slot so full-range tanh reads
            # finite values even where the trimmed matmuls never write
            pss0 = pss_pool.tile([128, 2, 512], F32, tag="pss")
            nc.vector.memset(pss0, 0.0)

            # ---------------- main loop: attention + interleaved work ------
            for c in range(NCH):
                nkv = 4 * (c + 1)
                for g in range(G):
                    st_next = 4 * (c + 1) + g + 1
                    if st_next < NST:
                        stage_q.append(lambda st=st_next: p1_mm(st))
                        stage_q.append(lambda st=st_next: p1_rope(st))
                        prefetch_xc(st_next + 1)
                    if g == 3 and c + 1 < NCH:
                        lo = 4 * (c + 1)
                        stage_q.append(lambda lo=lo: p1_rstd(lo, 4))
                        for st_f in range(lo, lo + 4):
                            stage_q.append(lambda st=st_f: p1_tr(st))

                    qT_c = qT_all[:, g, c * 512:(c + 1) * 512]
                    p_buf = p_bufs[(4 * c + g) % 2]
                    ps_y = psy_pool.tile([128, 512], F32, tag="psy")
                    ps_d = psd_pool.tile([128, 512], F32, tag="psd")

                    pairs = list(range(0, nkv, 2))
                    prev = None  # previous pair's (kb0, off_pair)
                    for kb0 in pairs:
                        offs = [max(0, 128 * (kb0 + i) - 512 * c) for i in (0, 1)]
                        off_p = offs[0]
                        ps_s = pss_pool.tile([128, 2, 512], F32, tag="pss")
                        for i in (0, 1):
                            kb = kb0 + i
                            nc.tensor.matmul(
                                ps_s[:, i, offs[i]:512],
                                kT_all[:, kb * 128:(kb + 1) * 128],
                                qT_c[:, offs[i]:512],
                                start=True, stop=True,
                            )
                        # V/d of the previous pair (deps resolved -> no stall)
                        if prev is not None:
                            for kb in (prev, prev + 1):
                                nc.tensor.matmul(ps_y, v_all[:, kb, :], p_buf[:, kb, :],
                                                 start=(kb == 0), stop=False)
                                nc.tensor.matmul(ps_d, ones_sq, p_buf[:, kb, :],
                                                 start=(kb == 0), stop=False)
                        emit_filler()
                        if kb0 == 0:
                            emit_filler()

                        # full-range tanh (stale PSUM is finite), exp from the
                        # pair's min offset; masks zero everything invalid
                        t_b = tb_pool.tile([128, 2, 512], F32, tag="tb")
                        nc.scalar.activation(t_b, ps_s,
                                             mybir.ActivationFunctionType.Tanh,
                                             scale=1.0 / CAP)
                        nc.scalar.activation(p_buf[:, kb0:kb0 + 2, off_p:512],
                                             t_b[:, :, off_p:512],
                                             mybir.ActivationFunctionType.Exp,
                                             scale=CAP)
                        for i in (0, 1):
                            kb = kb0 + i
                            r = kb - 4 * c
                            if r >= 0:
                                nc.vector.tensor_mul(p_buf[:, kb, off_p:512],
                                                     p_buf[:, kb, off_p:512],
                                                     masks[:, r, off_p:512])
                        prev = kb0

                    # tail: V/d of the last pair, then normalize
                    for kb in (prev, prev + 1):
                        nc.tensor.matmul(ps_y, v_all[:, kb, :], p_buf[:, kb, :],
                                         start=(kb == 0), stop=(kb == nkv - 1))
                        nc.tensor.matmul(ps_d, ones_sq, p_buf[:, kb, :],
                                         start=(kb == 0), stop=(kb == nkv - 1))
                    recip = rc_pool.tile([128, 512], F32, tag="rc")
                    nc.vector.reciprocal_approx_fast(out=recip, in_=ps_d)
                    nc.vector.tensor_mul(yT_all[:, g, c * 512:(c + 1) * 512],
                                         ps_y, recip)

                # queue output projection for this chunk, then drain any
                # leftover phase-1 stages with projection work interleaved
                for st in range(4 * c, 4 * c + 4):
                    for jc in range(4):
                        proj_q.append((st, jc))
                while stage_q:
                    stage_q.pop(0)()
                    if proj_q:
                        if evac_q:
                            proj_evac()
                        st, jc = proj_q.pop(0)
                        proj_mm(st, jc)

            # drain remaining projection units
            while proj_q or evac_q:
                emit_filler()

    nc.compile()
    return nc


def _host_prep(x, Wq, Wk, Wv, Wproj, q_gain):
    inv_freq = 1.0 / (ROPE_BASE ** (np.arange(0, ROPE, 2, dtype=np.float32) / ROPE))
    t = np.arange(S, dtype=np.float32)
    freqs = np.outer(t, inv_freq).astype(np.float32)  # [S, 16]
    cos = np.cos(freqs).astype(np.float32)
    sin = np.sin(freqs).astype(np.float32)
    cos4 = np.ascontiguousarray(np.tile(cos[:, None, :], (1, G, 1)).reshape(S, G * HALF))
    sin4 = np.ascontiguousarray(np.tile(sin[:, None, :], (1, G, 1)).reshape(S, G * HALF))

    xT = [np.ascontiguousarray(x[b].T) for b in range(B)]

    in_maps = []
    for core in range(8):
        b, h = core // HKV, core % HKV
        wqkv = np.ascontiguousarray(
            np.concatenate(
                [Wq[512 * h:512 * h + 512].T,
                 Wk[128 * h:128 * h + 128].T,
                 Wv[128 * h:128 * h + 128].T], axis=1
            )
        )
        wpT = np.ascontiguousarray(Wproj[:, 512 * h:512 * h + 512].T)
        gains = np.ascontiguousarray(
            np.broadcast_to((q_gain[G * h:G * h + G] / math.sqrt(HD)).astype(np.float32)[None, :],
                            (128, G))
        )
        in_maps.append({
            "xT": xT[b],
            "wqkv": wqkv.astype(np.float32),
            "wpT": wpT.astype(ml_dtypes.bfloat16),
            "gains": gains,
            "cos4": cos4,
            "sin4": sin4,
        })
    return in_maps


def kernel(x, Wq, Wk, Wv, Wproj, q_gain, _trace=False):
    x = np.asarray(x, dtype=np.float32)
    Wq = np.asarray(Wq, dtype=np.float32)
    Wk = np.asarray(Wk, dtype=np.float32)
    Wv = np.asarray(Wv, dtype=np.float32)
    Wproj = np.asarray(Wproj, dtype=np.float32)
    q_gain = np.asarray(q_gain, dtype=np.float32)

    if "nc" not in _CACHE:
        _CACHE["nc"] = _build()
    nc = _CACHE["nc"]

    in_maps = _host_prep(x, Wq, Wk, Wv, Wproj, q_gain)
    res = run_bass_kernel_spmd(nc, in_maps, core_ids=list(range(8)), trace=_trace)

    out = np.empty((B, S, D), dtype=np.float32)
    for b in range(B):
        acc = np.zeros((S, D), dtype=np.float64)
        for h in range(HKV):
            acc += res.results[b * HKV + h]["out"]
        out[b] = acc.astype(np.float32)
    if _trace:
        return out, res
    return out
